# revision 25
# baseline (speedup 1.0000x reference)
r"""Trainium2 Bass kernel for the triangular-DP "MAA layer" problem.

Reference computes, per frame t (T=1024, D=256, L=T+1 counts):
    q_t = (1-p_t) q_{t-1} + p_t shift(q_{t-1})          (Poisson-binomial DP)
    m_t = p_t a m_sh + (1-p_t) m + p_t b q_sh x_t       ([L, D] state)
    out = sum_i m_T[i, :]                               ([D])

Algebraic restructuring: the whole scan collapses to

    out[d] = sum_t c_t x[t, d],
    c_t    = p_t * I_t,   I_t = int_0^1 prod_{s != t} ((1-p_s) + p_s u) du.

The integrand is a boundary-layer spike at u=1 of width ~1/S, S = sum_s p_s.
Gauss-Legendre on the rescaled interval [1 - 30/S, 1] (host-computed from p)
converges at K=16 nodes to ~1e-12 (tail cut error e^-30).  With
f[t,k] = 1 + p_t (u_k - 1):

    slog_k = sum_t ln f[t,k]
    c_t    = p_t * sum_k exp(slog_k + ln w_k - ln f[t,k])
    out    = c^T @ x

Device mapping (t on partitions, 8 chunks of 128; k on free dim, K=16),
replicated on all 8 cores (collective latency floor exceeds compute):
  - one [128, 8+16+16] aux input carries pcol + host-pre-broadcast um1/lnw
    (no device-side row broadcasts at all), issued on the GpSimd engine's
    DMA queue so its packets flow in parallel with the x stream
  - the slog partition-reduce uses a [128,128] all-ones stationary so its
    PSUM output lands already broadcast across partitions
  - single big Ln and single big Exp on ScalarE; the Exp table load hides
    behind the DVE halving-adds + the slog matmul
  - final contraction: 8 accumulating bf16 PE matmuls (x shipped as bf16)
  - two junk matmuls lift the PE out of its lowest p-state early
"""

import numpy as np

T, D, NCH, P, K = 1024, 256, 8, 128, 8
N_CORES = 8

_CACHE = {}


def _build_program():
    import concourse.bass as bass
    import concourse.bacc as bacc
    import concourse.mybir as mybir
    import concourse.tile as tile

    f32 = mybir.dt.float32
    bf16 = mybir.dt.bfloat16
    A = mybir.AluOpType
    ACT = mybir.ActivationFunctionType

    nc = bacc.Bacc("TRN2", target_bir_lowering=False, debug=False,
                   num_devices=N_CORES)

    AUXW = NCH + 2 * K  # [pcol(8) | um1bc(16) | lnwbc(16)], host-broadcast
    paux_d = nc.dram_tensor("paux", [P, AUXW], f32, kind="ExternalInput")
    xa_d = nc.dram_tensor("xa", [P, NCH * D], bf16, kind="ExternalInput")
    out_d = nc.dram_tensor("out", [1, D], f32, kind="ExternalOutput")

    with tile.TileContext(nc) as tc:
        with (
            tc.tile_pool(name="sb", bufs=1) as sb,
            tc.tile_pool(name="ps", bufs=1, space=bass.MemorySpace.PSUM) as ps,
        ):
            paux = sb.tile([P, AUXW], f32, tag="paux")
            xa = sb.tile([P, NCH * D], bf16, tag="xa")
            # paux split across two engines' DMA queues so the small-packet
            # streams flow in parallel; xa second on Sync's queue
            nc.sync.dma_start(paux[0:64, :], paux_d[0:64, :])
            nc.scalar.dma_start(paux[64:P, :], paux_d[64:P, :])
            nc.sync.dma_start(xa[:], xa_d[:])
            pcol = paux[:, 0:NCH]
            um1bc = paux[:, NCH:NCH + K]
            lnwbc = paux[:, NCH + K:NCH + 2 * K]

            # PE warmup: junk matmuls lift the PE out of its lowest DVFS
            # p-state while the input DMAs land
            jmv = sb.tile([P, 512], bf16, tag="jmv")
            nc.gpsimd.memset(jmv[:], 0.0)
            onesbig = sb.tile([P, P], f32, tag="onesbig")
            nc.gpsimd.memset(onesbig[:], 1.0)
            jps = ps.tile([1, 512], f32, tag="jps")
            for _ in range(2):
                nc.tensor.matmul(jps[:], jmv[:, 0:1], jmv[:],
                                 start=True, stop=True)

            # fm1[t,(c,k)] = pcol[t,c] * um1[k]
            fm1 = sb.tile([P, NCH * K], f32, tag="fm1")
            um1_rep = um1bc.unsqueeze(1).broadcast_to([P, NCH, K])
            p_rep = pcol.unsqueeze(2).broadcast_to([P, NCH, K])
            nc.vector.tensor_tensor(fm1.rearrange("p (c k) -> p c k", c=NCH),
                                    um1_rep, p_rep, op=A.mult)

            slogbc_ps = ps.tile([P, K], f32, tag="slogbc_ps")

            # lf = Ln(fm1 + 1)
            lfbig = sb.tile([P, NCH * K], f32, tag="lfbig")
            nc.scalar.activation(lfbig[:], fm1[:], ACT.Ln, bias=1.0)

            # lfsum[t, k] = sum_c lf[t,(c,k)] : one strided reduce (at K=8
            # the strided read is only 64 elements/partition, so one op
            # beats three halving adds' fixed overheads)
            lfsum = sb.tile([P, K], f32, tag="lfsum")
            nc.vector.tensor_reduce(
                lfsum[:], lfbig.rearrange("p (c k) -> p k c", c=NCH),
                axis=mybir.AxisListType.X, op=A.add)

            # slogbc[i, k] = sum_t lfsum[t, k] for every i: the all-ones
            # stationary makes the PE reduce land pre-broadcast in PSUM
            nc.tensor.matmul(slogbc_ps[:], onesbig[:], lfsum[:],
                             start=True, stop=True)

            # w2 = slog + lnw ; arg = w2 - lf ; e = exp(arg)
            w2 = sb.tile([P, K], f32, tag="w2")
            nc.vector.tensor_tensor(w2[:], slogbc_ps[:], lnwbc, op=A.add)
            arg = sb.tile([P, NCH * K], f32, tag="arg")
            w2_rep = w2.unsqueeze(1).broadcast_to([P, NCH, K])
            nc.vector.tensor_tensor(arg.rearrange("p (c k) -> p c k", c=NCH),
                                    w2_rep,
                                    lfbig.rearrange("p (c k) -> p c k", c=NCH),
                                    op=A.subtract)
            # Exp -> k-reduce -> p-mult -> matmuls, pipelined in two 4-chunk
            # halves with SEPARATE psum accumulation groups (a semaphore wait
            # inside an open group is not safe), summed at the end on DVE
            HC = NCH // 2
            e = sb.tile([P, NCH * K], f32, tag="e")
            cfin8 = sb.tile([P, NCH], f32, tag="cfin8")
            cfinb = sb.tile([P, NCH], bf16, tag="cfinb")
            out_ps = ps.tile([1, D], f32, tag="out_ps")
            for h in range(2):
                cs, ce = h * HC, (h + 1) * HC
                ks, ke = cs * K, ce * K
                nc.scalar.activation(e[:, ks:ke], arg[:, ks:ke], ACT.Exp)
                nc.vector.tensor_reduce(
                    cfin8[:, cs:ce],
                    e[:, ks:ke].rearrange("p (c k) -> p c k", c=HC),
                    axis=mybir.AxisListType.X, op=A.add)
                nc.vector.tensor_tensor(cfinb[:, cs:ce], cfin8[:, cs:ce],
                                        pcol[:, cs:ce], op=A.mult)
                for c in range(cs, ce):
                    nc.tensor.matmul(out_ps[:], cfinb[:, c:c + 1],
                                     xa[:, c * D:(c + 1) * D],
                                     start=(c == 0), stop=(c == NCH - 1),
                                     skip_group_check=True)
            out_sb = sb.tile([1, D], f32, tag="outsb")
            nc.vector.tensor_copy(out_sb[:], out_ps[:])
            nc.sync.dma_start(out_d[:], out_sb[:])

    nc.compile()
    return nc


def _make_in_map(p, x):
    import ml_dtypes

    p = np.ascontiguousarray(np.asarray(p, dtype=np.float32)).reshape(T)
    x = np.ascontiguousarray(np.asarray(x, dtype=np.float32)).reshape(T, D)
    S = float(np.sum(np.asarray(p, np.float64)))
    delta = min(1.0, 30.0 / max(S, 1.0))
    nodes, weights = np.polynomial.legendre.leggauss(K)
    u = 1.0 - delta + delta * (nodes + 1.0) * 0.5
    w = weights * delta * 0.5
    paux = np.empty((P, NCH + 2 * K), np.float32)
    paux[:, 0:NCH] = p.reshape(NCH, P).T
    paux[:, NCH:NCH + K] = (u - 1.0).astype(np.float32)[None, :]
    paux[:, NCH + K:NCH + 2 * K] = np.log(w).astype(np.float32)[None, :]
    xa = np.ascontiguousarray(
        x.reshape(NCH, P, D).transpose(1, 0, 2).reshape(P, NCH * D)
    ).astype(ml_dtypes.bfloat16)
    return {"paux": paux, "xa": xa}


def _run(p, x, trace=False, tmpdir=None):
    from concourse.bass_utils import run_bass_kernel_spmd

    if "nc" not in _CACHE:
        _CACHE["nc"] = _build_program()
    nc = _CACHE["nc"]
    in_map = _make_in_map(p, x)
    in_maps = [in_map for _ in range(N_CORES)]
    res = run_bass_kernel_spmd(nc, in_maps, list(range(N_CORES)),
                               trace=trace, tmpdir=tmpdir)
    out = np.asarray(res.results[0]["out"], dtype=np.float32).reshape(D)
    return out, res


def kernel(p, x):
    out, _ = _run(p, x, trace=False)
    return out


# revision 26
# speedup vs baseline: 1.0316x; 1.0316x over previous
r"""Trainium2 Bass kernel for the triangular-DP "MAA layer" problem.

Reference computes, per frame t (T=1024, D=256, L=T+1 counts):
    q_t = (1-p_t) q_{t-1} + p_t shift(q_{t-1})          (Poisson-binomial DP)
    m_t = p_t a m_sh + (1-p_t) m + p_t b q_sh x_t       ([L, D] state)
    out = sum_i m_T[i, :]                               ([D])

Algebraic restructuring: the whole scan collapses to

    out[d] = sum_t c_t x[t, d],
    c_t    = p_t * I_t,   I_t = int_0^1 prod_{s != t} ((1-p_s) + p_s u) du.

The integrand is a boundary-layer spike at u=1 of width ~1/S, S = sum_s p_s.
Gauss-Legendre on the rescaled interval [1 - 30/S, 1] (host-computed from p)
converges at K=16 nodes to ~1e-12 (tail cut error e^-30).  With
f[t,k] = 1 + p_t (u_k - 1):

    slog_k = sum_t ln f[t,k]
    c_t    = p_t * sum_k exp(slog_k + ln w_k - ln f[t,k])
    out    = c^T @ x

Device mapping (t on partitions, 8 chunks of 128; k on free dim, K=16),
replicated on all 8 cores (collective latency floor exceeds compute):
  - one [128, 8+16+16] aux input carries pcol + host-pre-broadcast um1/lnw
    (no device-side row broadcasts at all), issued on the GpSimd engine's
    DMA queue so its packets flow in parallel with the x stream
  - the slog partition-reduce uses a [128,128] all-ones stationary so its
    PSUM output lands already broadcast across partitions
  - single big Ln and single big Exp on ScalarE; the Exp table load hides
    behind the DVE halving-adds + the slog matmul
  - final contraction: 8 accumulating bf16 PE matmuls (x shipped as bf16)
  - two junk matmuls lift the PE out of its lowest p-state early
"""

import numpy as np

T, D, NCH, P, K = 1024, 256, 8, 128, 8
N_CORES = 8

_CACHE = {}


def _build_program():
    import concourse.bass as bass
    import concourse.bacc as bacc
    import concourse.mybir as mybir
    import concourse.tile as tile

    f32 = mybir.dt.float32
    bf16 = mybir.dt.bfloat16
    A = mybir.AluOpType
    ACT = mybir.ActivationFunctionType

    nc = bacc.Bacc("TRN2", target_bir_lowering=False, debug=False,
                   num_devices=N_CORES)

    AUXW = NCH + 2 * K  # [pcol(8) | um1bc(16) | lnwbc(16)], host-broadcast
    paux_d = nc.dram_tensor("paux", [P, AUXW], f32, kind="ExternalInput")
    xa_d = nc.dram_tensor("xa", [P, NCH * D], bf16, kind="ExternalInput")
    out_d = nc.dram_tensor("out", [1, D], f32, kind="ExternalOutput")

    with tile.TileContext(nc) as tc:
        with (
            tc.tile_pool(name="sb", bufs=1) as sb,
            tc.tile_pool(name="ps", bufs=1, space=bass.MemorySpace.PSUM) as ps,
        ):
            paux = sb.tile([P, AUXW], f32, tag="paux")
            xa = sb.tile([P, NCH * D], bf16, tag="xa")
            # paux split across two engines' DMA queues so the small-packet
            # streams flow in parallel; xa second on Sync's queue
            nc.sync.dma_start(paux[0:64, :], paux_d[0:64, :])
            nc.scalar.dma_start(paux[64:P, :], paux_d[64:P, :])
            nc.sync.dma_start(xa[:], xa_d[:])
            pcol = paux[:, 0:NCH]
            um1bc = paux[:, NCH:NCH + K]
            lnwbc = paux[:, NCH + K:NCH + 2 * K]

            # PE warmup: junk matmuls lift the PE out of its lowest DVFS
            # p-state while the input DMAs land
            jmv = sb.tile([P, 512], bf16, tag="jmv")
            nc.gpsimd.memset(jmv[:], 0.0)
            onesbig = sb.tile([P, P], f32, tag="onesbig")
            nc.gpsimd.memset(onesbig[:], 1.0)
            jps = ps.tile([1, 512], f32, tag="jps")
            for _ in range(2):
                nc.tensor.matmul(jps[:], jmv[:, 0:1], jmv[:],
                                 start=True, stop=True)

            # fm1[t,(c,k)] = pcol[t,c] * um1[k]
            fm1 = sb.tile([P, NCH * K], f32, tag="fm1")
            um1_rep = um1bc.unsqueeze(1).broadcast_to([P, NCH, K])
            p_rep = pcol.unsqueeze(2).broadcast_to([P, NCH, K])
            nc.vector.tensor_tensor(fm1.rearrange("p (c k) -> p c k", c=NCH),
                                    um1_rep, p_rep, op=A.mult)

            slogbc_ps = ps.tile([P, K], f32, tag="slogbc_ps")

            # lf = Ln(fm1 + 1)
            lfbig = sb.tile([P, NCH * K], f32, tag="lfbig")
            nc.scalar.activation(lfbig[:], fm1[:], ACT.Ln, bias=1.0)

            # lfsum[t, k] = sum_c lf[t,(c,k)] : one strided reduce (at K=8
            # the strided read is only 64 elements/partition, so one op
            # beats three halving adds' fixed overheads)
            lfsum = sb.tile([P, K], f32, tag="lfsum")
            nc.vector.tensor_reduce(
                lfsum[:], lfbig.rearrange("p (c k) -> p k c", c=NCH),
                axis=mybir.AxisListType.X, op=A.add)

            # lnw preload goes into the slog PSUM first (copy emitted after
            # fm1 on the DVE queue); the all-ones-stationary partition-reduce
            # then accumulates on top, landing slog+lnw pre-broadcast
            nc.vector.tensor_copy(slogbc_ps[:], lnwbc)
            nc.tensor.matmul(slogbc_ps[:], onesbig[:], lfsum[:],
                             start=False, stop=True, skip_group_check=True)

            # arg = (slog + lnw) - lf ; e = exp(arg)
            arg = sb.tile([P, NCH * K], f32, tag="arg")
            w2_rep = slogbc_ps.unsqueeze(1).broadcast_to([P, NCH, K])
            nc.vector.tensor_tensor(arg.rearrange("p (c k) -> p c k", c=NCH),
                                    w2_rep,
                                    lfbig.rearrange("p (c k) -> p c k", c=NCH),
                                    op=A.subtract)
            # Exp -> k-reduce -> p-mult -> matmuls, pipelined in two 4-chunk
            # halves with SEPARATE psum accumulation groups (a semaphore wait
            # inside an open group is not safe), summed at the end on DVE
            HC = NCH // 2
            e = sb.tile([P, NCH * K], f32, tag="e")
            cfin8 = sb.tile([P, NCH], f32, tag="cfin8")
            cfinb = sb.tile([P, NCH], bf16, tag="cfinb")
            out_ps = ps.tile([1, D], f32, tag="out_ps")
            for h in range(2):
                cs, ce = h * HC, (h + 1) * HC
                ks, ke = cs * K, ce * K
                nc.scalar.activation(e[:, ks:ke], arg[:, ks:ke], ACT.Exp)
                nc.vector.tensor_reduce(
                    cfin8[:, cs:ce],
                    e[:, ks:ke].rearrange("p (c k) -> p c k", c=HC),
                    axis=mybir.AxisListType.X, op=A.add)
                nc.vector.tensor_tensor(cfinb[:, cs:ce], cfin8[:, cs:ce],
                                        pcol[:, cs:ce], op=A.mult)
                for c in range(cs, ce):
                    nc.tensor.matmul(out_ps[:], cfinb[:, c:c + 1],
                                     xa[:, c * D:(c + 1) * D],
                                     start=(c == 0), stop=(c == NCH - 1),
                                     skip_group_check=True)
            out_sb = sb.tile([1, D], f32, tag="outsb")
            nc.vector.tensor_copy(out_sb[:], out_ps[:])
            nc.sync.dma_start(out_d[:], out_sb[:])

    nc.compile()
    return nc


def _make_in_map(p, x):
    import ml_dtypes

    p = np.ascontiguousarray(np.asarray(p, dtype=np.float32)).reshape(T)
    x = np.ascontiguousarray(np.asarray(x, dtype=np.float32)).reshape(T, D)
    S = float(np.sum(np.asarray(p, np.float64)))
    delta = min(1.0, 30.0 / max(S, 1.0))
    nodes, weights = np.polynomial.legendre.leggauss(K)
    u = 1.0 - delta + delta * (nodes + 1.0) * 0.5
    w = weights * delta * 0.5
    paux = np.empty((P, NCH + 2 * K), np.float32)
    paux[:, 0:NCH] = p.reshape(NCH, P).T
    paux[:, NCH:NCH + K] = (u - 1.0).astype(np.float32)[None, :]
    paux[:, NCH + K:NCH + 2 * K] = np.log(w).astype(np.float32)[None, :]
    xa = np.ascontiguousarray(
        x.reshape(NCH, P, D).transpose(1, 0, 2).reshape(P, NCH * D)
    ).astype(ml_dtypes.bfloat16)
    return {"paux": paux, "xa": xa}


def _run(p, x, trace=False, tmpdir=None):
    from concourse.bass_utils import run_bass_kernel_spmd

    if "nc" not in _CACHE:
        _CACHE["nc"] = _build_program()
    nc = _CACHE["nc"]
    in_map = _make_in_map(p, x)
    in_maps = [in_map for _ in range(N_CORES)]
    res = run_bass_kernel_spmd(nc, in_maps, list(range(N_CORES)),
                               trace=trace, tmpdir=tmpdir)
    out = np.asarray(res.results[0]["out"], dtype=np.float32).reshape(D)
    return out, res


def kernel(p, x):
    out, _ = _run(p, x, trace=False)
    return out
